# revision 16
# baseline (speedup 1.0000x reference)
"""DistMult edge scoring on 8 Trainium2 NeuronCores.

score[e] = sum_d node_emb[src[e], d] * rel_emb[e, d] * node_emb[dst[e], d]

Strategy (data-parallel over edges, node table replicated per core):
  - Each of the 8 cores gets the full node_emb table in its HBM plus a
    1/8 shard of the edges (rel rows + src/dst indices).
  - The node table is split into 4 blocks of 25000 rows. Each core's
    edges are bucketed host-side into 16 groups by (src_block,
    dst_block); within a group both gathers address a 25000-row window
    so GPSIMD dma_gather with int16 local indices applies.
  - KEY PERF FACT (measured): dma_gather throughput is limited by
    per-queue SWDGE descriptor generation at ~8.3 ns/row; queues scale
    perfectly, so the module allocates 4 SWDGE queues and round-robins
    gather instructions across them (~2.07 ns/row aggregate).  Element
    size does not matter (512B f32 rows gather at the same rows/s as
    256B bf16 rows), so the node table stays f32/exact.
  - The whole datapath is bf16 (node table + rel pre-converted
    host-side): halves HBM traffic and doubles DVE throughput; the f32
    blocked reduce keeps accumulation exact enough (l2 ~4e-3).
    Uniform dtypes only — mixed-dtype DVE ops crash silicon.
  - One gather pair per group (CH=5120) minimizes Pool-engine dispatch
    overhead; group slots zero-padded to CAP (rel rows zeroed there).
  - Host pre-permutes rel rows into the slot layout (zeros at padding)
    and inverts the permutation on the returned score plane.
"""

import os

import numpy as np

N_NODES = 100000
E_TOTAL = 600000
D = 128
N_CORES = 8
E_CORE = E_TOTAL // N_CORES  # 75000

NB = 4                # node blocks
BS = N_NODES // NB    # block size (rows per gather window)
G = NB * NB           # groups per core
CH = int(os.environ.get("KNL_CH", "5120"))  # edge slots per chunk
CAP = int(os.environ.get("KNL_CAP", "5120"))  # slots per group
S = G * CAP           # total slots per core
COLS = S // 128       # score plane columns

NQ = int(os.environ.get("KNL_NQ", "4"))          # SWDGE queues
# Stability (HW-validated): runtime count registers, trailing-negative
# index padding, and mixed-dtype (f32*bf16) DVE multiplies each cause
# intermittent device crashes on silicon (NRT_EXEC_UNIT_UNRECOVERABLE)
# despite passing CoreSim.  Keep all three OFF.
USE_CNT_REG = os.environ.get("KNL_CNTREG", "0") == "1"
USE_NEG_PAD = os.environ.get("KNL_NEGPAD", "0") == "1"
REL_F32 = os.environ.get("KNL_RELF32", "1") == "1"
ALL_BF16 = os.environ.get("KNL_BF16", "1") == "1"

_CACHE: dict = {}


def _build_module(repeats: int = 1):
    import concourse.bacc as bacc
    import concourse.mybir as mybir
    from concourse.tile import TileContext

    nc = bacc.Bacc(
        "TRN2",
        debug=False,
        enable_asserts=False,
        target_bir_lowering=False,
        num_devices=N_CORES,
        num_swdge_queues=NQ,
        dynamic_dma_scratch_size=int(os.environ.get("KNL_SCRATCH", "16384")),
    )
    f32 = mybir.dt.float32
    bf16 = mybir.dt.bfloat16
    i16 = mybir.dt.int16
    i32 = mybir.dt.int32

    ndt = bf16 if ALL_BF16 else f32
    node = nc.dram_tensor("node_emb", [N_NODES, D], ndt, kind="ExternalInput").ap()
    reldt = ndt if ALL_BF16 else (f32 if REL_F32 else bf16)
    relsw = nc.dram_tensor("relsw", [128, S], reldt, kind="ExternalInput").ap()
    srci = nc.dram_tensor("srci", [128, S // 16], i16, kind="ExternalInput").ap()
    dsti = nc.dram_tensor("dsti", [128, S // 16], i16, kind="ExternalInput").ap()
    cnt1 = nc.dram_tensor("cnt1", [1, G], i32, kind="ExternalInput").ap()
    out = nc.dram_tensor("scores", [128, COLS], f32, kind="ExternalOutput").ap()

    n_chunks = CAP // CH

    with TileContext(nc) as tc:
        with (
            tc.tile_pool(name="idx", bufs=1) as idxp,
            tc.tile_pool(name="ht", bufs=6) as htp,
            tc.tile_pool(name="rl", bufs=4) as rlp,
            tc.tile_pool(name="res", bufs=1) as resp,
        ):
            src_t = idxp.tile([128, S // 16], i16, tag="srci")
            dst_t = idxp.tile([128, S // 16], i16, tag="dsti")
            cnt_t = idxp.tile([1, G], i32, tag="cnt1")
            score_t = resp.tile([128, COLS], f32, tag="score")
            nc.sync.dma_start(out=src_t[:], in_=srci[:])
            nc.sync.dma_start(out=dst_t[:], in_=dsti[:])
            nc.sync.dma_start(out=cnt_t[:], in_=cnt1[:])

            # per-group count of real edges in the LAST chunk (runtime).
            if USE_CNT_REG:
                cnt_regs = [
                    nc.gpsimd.value_load(
                        cnt_t[0:1, g : g + 1], min_val=1, max_val=CH
                    )
                    for g in range(G)
                ]
            else:
                cnt_regs = [CH] * G

            qi = 0
            for _rep in range(repeats):
              for g in range(G):
                sb = (g // NB) * BS
                db = (g % NB) * BS
                for c in range(n_chunks):
                    s0 = g * CAP + c * CH
                    nreg = cnt_regs[g] if c == n_chunks - 1 else CH
                    head = htp.tile([128, CH], ndt, tag="head")
                    tail = htp.tile([128, CH], ndt, tag="tail")
                    relt = rlp.tile([128, CH], reldt, tag="rel")
                    nc.gpsimd.dma_gather(
                        out_ap=head[:].rearrange("p (c d) -> p c d", d=D),
                        in_ap=node[sb : sb + BS],
                        idxs_ap=src_t[:, s0 // 16 : (s0 + CH) // 16],
                        num_idxs=CH,
                        num_idxs_reg=nreg,
                        elem_size=D,
                        single_packet=False,
                        queue_num=qi % NQ,
                    )
                    qi += 1
                    nc.gpsimd.dma_gather(
                        out_ap=tail[:].rearrange("p (c d) -> p c d", d=D),
                        in_ap=node[db : db + BS],
                        idxs_ap=dst_t[:, s0 // 16 : (s0 + CH) // 16],
                        num_idxs=CH,
                        num_idxs_reg=nreg,
                        elem_size=D,
                        single_packet=False,
                        queue_num=qi % NQ,
                    )
                    qi += 1
                    nc.sync.dma_start(out=relt[:], in_=relsw[:, s0 : s0 + CH])
                    nc.vector.tensor_tensor(
                        out=head[:], in0=head[:], in1=relt[:],
                        op=mybir.AluOpType.mult,
                    )
                    nc.vector.tensor_tensor(
                        out=head[:], in0=head[:], in1=tail[:],
                        op=mybir.AluOpType.mult,
                    )
                    nc.vector.tensor_reduce(
                        out=score_t[:, s0 // 128 : (s0 + CH) // 128],
                        in_=head[:].rearrange("p (c d) -> p c d", d=D),
                        axis=mybir.AxisListType.X,
                        op=mybir.AluOpType.add,
                    )

            nc.sync.dma_start(out=out[:], in_=score_t[:])

    nc.compile()
    _align_gather_queues(nc)
    return nc


def _align_gather_queues(nc):
    """Rewrite each gather's queue_num to (its DMASW sem lane) % NQ.

    The Tile scheduler assigns DMA-completion semaphores round-robin over 8
    DMASW lanes in *scheduled* order, which differs from emission order; a
    semaphore shared by two SWDGE queues breaks the FIFO-completion
    assumption (and the simulator rejects it).  Aligning queue to lane
    guarantees one queue per semaphore and still round-robins the 4 queues.
    """
    import re

    for blk in nc.m.functions[0].blocks:
        for inst in blk.instructions:
            if type(inst).__name__ == "InstDMAGatherAnt":
                u = inst.sync_info.on_update[0]
                m = re.match(r"DMASW(\d+)_", u.ant_name)
                assert m, f"gather sem not on a DMASW lane: {u.ant_name}"
                inst.queue_num = int(m.group(1)) % NQ


def _get_module(repeats: int = 1):
    key = ("nc", repeats)
    if key not in _CACHE:
        _CACHE[key] = _build_module(repeats)
    return _CACHE[key]


def _wrap16(x: np.ndarray) -> np.ndarray:
    """[S] int16 -> [128, S/16] gather index plane (16-wrap, replicated 8x)."""
    w = x.reshape(S // 16, 16).T
    return np.ascontiguousarray(np.tile(w, (8, 1)))


def _prep_core(rel_c, src_c, dst_c):
    import ml_dtypes

    src_c = src_c.astype(np.int64)
    dst_c = dst_c.astype(np.int64)
    g = (src_c // BS) * NB + (dst_c // BS)
    order = np.argsort(g, kind="stable")
    gs = g[order]
    counts = np.bincount(g, minlength=G)
    if counts.max() > CAP:
        raise ValueError(f"group overflow: {counts.max()} > CAP={CAP}")
    if USE_NEG_PAD and counts.min() <= CH:
        # trailing-negative trim requires every non-final chunk to be full
        raise ValueError(f"group underflow: {counts.min()} <= CH={CH}")
    cum = np.zeros(G, dtype=np.int64)
    cum[1:] = np.cumsum(counts)[:-1]
    rank = np.arange(E_CORE) - cum[gs]
    slots = gs * CAP + rank  # slot for each sorted edge

    pad = -1 if USE_NEG_PAD else 0
    loc_src = np.full(S, pad, dtype=np.int16)
    loc_dst = np.full(S, pad, dtype=np.int16)
    loc_src[slots] = (src_c[order] - (gs // NB) * BS).astype(np.int16)
    loc_dst[slots] = (dst_c[order] - (gs % NB) * BS).astype(np.int16)

    rel_perm = np.zeros((S, D), dtype=np.float32)
    rel_perm[slots] = rel_c[order]
    relsw = np.ascontiguousarray(
        rel_perm.reshape(S // 128, 128, D).transpose(1, 0, 2).reshape(128, S)
    )
    if ALL_BF16 or not REL_F32:
        relsw = relsw.astype(ml_dtypes.bfloat16)
    cnt1 = (counts - CH).astype(np.int32).reshape(1, G)
    return (
        {
            "relsw": relsw,
            "srci": _wrap16(loc_src),
            "dsti": _wrap16(loc_dst),
            "cnt1": cnt1,
        },
        order,
        slots,
    )


def make_in_maps(node_emb, rel_emb, src, dst):
    node = np.ascontiguousarray(np.asarray(node_emb, dtype=np.float32))
    if ALL_BF16:
        import ml_dtypes
        node = np.ascontiguousarray(node.astype(ml_dtypes.bfloat16))
    rel_emb = np.asarray(rel_emb, dtype=np.float32)
    src = np.asarray(src)
    dst = np.asarray(dst)
    in_maps, metas = [], []
    for c in range(N_CORES):
        sl = slice(c * E_CORE, (c + 1) * E_CORE)
        m, order, slots = _prep_core(rel_emb[sl], src[sl], dst[sl])
        m["node_emb"] = node
        in_maps.append(m)
        metas.append((order, slots))
    return in_maps, metas


def gather_outputs(results, metas) -> np.ndarray:
    scores = np.empty(E_TOTAL, dtype=np.float32)
    for c in range(N_CORES):
        plane = np.asarray(results[c]["scores"], dtype=np.float32)  # [128, COLS]
        lin = plane.T.ravel()  # lin[slot], slot = col*128 + p
        order, slots = metas[c]
        out_c = np.empty(E_CORE, dtype=np.float32)
        out_c[order] = lin[slots]
        scores[c * E_CORE : (c + 1) * E_CORE] = out_c
    return scores


def run(node_emb, rel_emb, src, dst, trace=False):
    from concourse import bass_utils
    from concourse.bass_interp import get_hw_module

    nc = _get_module()
    in_maps, metas = make_in_maps(node_emb, rel_emb, src, dst)
    old_m = nc.m
    nc.m = get_hw_module(nc.m)
    try:
        res = bass_utils.run_bass_kernel_spmd(
            nc, in_maps, core_ids=list(range(N_CORES)), trace=trace
        )
    finally:
        nc.m = old_m
    return gather_outputs(res.results, metas), res


def kernel(node_emb, rel_emb, src, dst):
    scores, _ = run(node_emb, rel_emb, src, dst, trace=False)
    return scores
